# revision 17
# baseline (speedup 1.0000x reference)
"""Multi-head attention (B=2, S=2048, H=16, D=64) on 8 TRN2 NeuronCores.

Sharding: data parallel on batch (2) x tensor parallel on heads (16 -> 4 per
core).  Core c handles batch c//4 and heads [4*(c%4), 4*(c%4)+4).  Each core
projects q/k/v for its head group from its batch's activations, runs the
full S x S attention for its 4 heads, and writes ctx in [head, D, S] layout.
The host transposes/concatenates shards (not part of HW exec time).

Device kernel (per core, identical SPMD program, no collectives):
  - qT/kT computed directly in [D, S] layout (head pairs packed into 128
    partitions) so the scores matmul needs no transposes.
  - scoresT tiles [S_k=128, S_q] = kT_chunk.T @ qT; softmax denominator via a
    ones-column appended to v (one matmul stream produces ctx and denom).
  - exp on the scalar engine with the 1/sqrt(D) scale folded in; bf16
    matmul operands, f32 PSUM accumulation; final normalize = DVE divide.
  - padding mask folded into v_aug row zeroing (exp(x-1e4) underflows to 0
    in f32, so zeroing masked key rows is exactly equivalent).
"""

import numpy as np
import ml_dtypes

import concourse.bass as bass
import concourse.tile as tile
from concourse import bacc, mybir
from concourse.bass_utils import run_bass_kernel_spmd

B, S, H, D = 2, 2048, 16, 64
HID = H * D
NCORES = 8
HPC = 4               # heads per core
COLS = HPC * D        # 256 projection columns per core
KC = HID // 128       # 8 contraction chunks for projections
QC = S // 512         # 4 query chunks of 512
MC = S // 128         # 16 key chunks of 128

BF16 = mybir.dt.bfloat16
F32 = mybir.dt.float32
np_bf16 = ml_dtypes.bfloat16

_CACHE = {}


def build(apply_mask: bool) -> bass.Bass:
    nc = bacc.Bacc(None, target_bir_lowering=False, debug=False)

    xT = nc.declare_dram_parameter("xT", [HID, S], BF16, isOutput=False)
    wq = nc.declare_dram_parameter("wq", [HID, COLS], BF16, isOutput=False)
    wk = nc.declare_dram_parameter("wk", [HID, COLS], BF16, isOutput=False)
    wv = nc.declare_dram_parameter("wv", [HID, COLS], BF16, isOutput=False)
    bq = nc.declare_dram_parameter("bq", [128, 2], F32, isOutput=False)
    bk = nc.declare_dram_parameter("bk", [128, 2], F32, isOutput=False)
    bv = nc.declare_dram_parameter("bv", [128, COLS], F32, isOutput=False)
    if apply_mask:
        mm_in = nc.declare_dram_parameter("maskm", [128, MC], F32, isOutput=False)
    out_ext = nc.declare_dram_parameter("out", [HPC, D, S], F32, isOutput=True)

    with tile.TileContext(nc) as tc:
        with (
            tc.tile_pool(name="singles", bufs=1) as singles,
            tc.tile_pool(name="work", bufs=3) as work,
            tc.tile_pool(name="psum", bufs=2, space="PSUM") as psum,
        ):
            # ---- input DMA ----
            x_sb = singles.tile([128, KC, S], BF16)
            for kc in range(KC):
                nc.sync.dma_start(out=x_sb[:, kc, :], in_=xT[kc * 128:(kc + 1) * 128, :])

            wq_sb = singles.tile([128, KC, COLS], BF16)
            wk_sb = singles.tile([128, KC, COLS], BF16)
            wv_sb = singles.tile([128, KC, COLS], BF16)
            for w_sb, w_in in ((wq_sb, wq), (wk_sb, wk), (wv_sb, wv)):
                for kc in range(KC):
                    nc.sync.dma_start(out=w_sb[:, kc, :], in_=w_in[kc * 128:(kc + 1) * 128, :])

            bq_sb = singles.tile([128, 2], F32)
            nc.sync.dma_start(out=bq_sb, in_=bq[:, :])
            bk_sb = singles.tile([128, 2], F32)
            nc.sync.dma_start(out=bk_sb, in_=bk[:, :])
            bv_sb = singles.tile([128, COLS], F32)
            nc.sync.dma_start(out=bv_sb, in_=bv[:, :])
            if apply_mask:
                mm_sb = singles.tile([128, MC], F32)
                nc.sync.dma_start(out=mm_sb, in_=mm_in[:, :])

            # ---- projections ----
            # qT/kT: [128, pair, S]; partitions 0:64 = head 2p, 64:128 = head 2p+1
            qT = singles.tile([128, 2, S], BF16)
            kT = singles.tile([128, 2, S], BF16)
            # v_aug: [128, key_chunk, head, 128]; cols 64:128 are ones columns,
            # so the ctx matmul emits the softmax denominator replicated into
            # psum partitions 64:128 at no extra cost (matmul cost is N-bound)
            v_aug = singles.tile([128, MC, HPC, 128], BF16)
            nc.vector.memset(v_aug[:, :, :, 64:128], 1.0)

            def project_T(dst, w_sb, b_sb, p):
                for qc in range(QC):
                    ps = psum.tile([128, 512], F32, tag="proj_ps", name=f"pt_{nc.next_id()}")
                    for kc in range(KC):
                        nc.tensor.matmul(
                            ps,
                            lhsT=w_sb[:, kc, p * 128:(p + 1) * 128],
                            rhs=x_sb[:, kc, qc * 512:(qc + 1) * 512],
                            start=(kc == 0), stop=(kc == KC - 1),
                        )
                    nc.vector.tensor_tensor(
                        out=dst[:, p, qc * 512:(qc + 1) * 512],
                        in0=ps,
                        in1=b_sb[:, p:p + 1].to_broadcast([128, 512]),
                        op=mybir.AluOpType.add,
                    )

            def project_v():
                for mc in range(MC):
                    ps = psum.tile([128, COLS], F32, tag="proj_ps", name=f"pv_{nc.next_id()}")
                    for kc in range(KC):
                        nc.tensor.matmul(
                            ps,
                            lhsT=x_sb[:, kc, mc * 128:(mc + 1) * 128],
                            rhs=wv_sb[:, kc, :],
                            start=(kc == 0), stop=(kc == KC - 1),
                        )
                    nc.vector.tensor_tensor(
                        out=v_aug[:, mc, :, 0:64],
                        in0=ps[:, :].rearrange("p (h d) -> p h d", h=HPC),
                        in1=bv_sb.rearrange("p (h d) -> p h d", h=HPC),
                        op=mybir.AluOpType.add,
                    )
                    if apply_mask:
                        nc.vector.tensor_tensor(
                            out=v_aug[:, mc, :, :],
                            in0=v_aug[:, mc, :, :],
                            in1=mm_sb[:, mc:mc + 1, None].to_broadcast([128, HPC, 128]),
                            op=mybir.AluOpType.mult,
                        )

            def attention(p):
                ha, hb = 2 * p, 2 * p + 1
                for qc in range(QC):
                    qsl = slice(qc * 512, (qc + 1) * 512)
                    ctx_a = psum.tile([128, 512], F32, tag="ctx", name=f"ca_{nc.next_id()}")
                    ctx_b = psum.tile([128, 512], F32, tag="ctx", name=f"cb_{nc.next_id()}")
                    for kc2 in range(MC // 2):
                        kc0, kc1 = 2 * kc2, 2 * kc2 + 1
                        s_a = psum.tile([128, 1024], F32, tag="sps", name=f"sa_{nc.next_id()}")
                        s_b = psum.tile([128, 1024], F32, tag="sps", name=f"sb_{nc.next_id()}")
                        # paired row-group matmuls: head a on array rows 0:63,
                        # head b on rows 64:127 run concurrently
                        for i, kc in enumerate((kc0, kc1)):
                            ksl = slice(kc * 128, (kc + 1) * 128)
                            csl = slice(i * 512, (i + 1) * 512)
                            nc.tensor.matmul(
                                s_a[:, csl], lhsT=kT[0:64, p, ksl], rhs=qT[0:64, p, qsl],
                                start=True, stop=True)
                            nc.tensor.matmul(
                                s_b[:, csl], lhsT=kT[64:128, p, ksl], rhs=qT[64:128, p, qsl],
                                start=True, stop=True)
                        e_a = work.tile([128, 1024], BF16, tag="expT", name=f"ea_{nc.next_id()}")
                        e_b = work.tile([128, 1024], BF16, tag="expT", name=f"eb_{nc.next_id()}")
                        nc.scalar.activation(e_a, s_a, mybir.ActivationFunctionType.Exp,
                                             scale=0.125)
                        nc.scalar.activation(e_b, s_b, mybir.ActivationFunctionType.Exp,
                                             scale=0.125)
                        for i, kc in enumerate((kc0, kc1)):
                            csl = slice(i * 512, (i + 1) * 512)
                            nc.tensor.matmul(
                                ctx_a, lhsT=v_aug[:, kc, ha, :], rhs=e_a[:, csl],
                                start=(kc == 0), stop=(kc == MC - 1))
                            nc.tensor.matmul(
                                ctx_b, lhsT=v_aug[:, kc, hb, :], rhs=e_b[:, csl],
                                start=(kc == 0), stop=(kc == MC - 1))
                    for h, ctx in ((ha, ctx_a), (hb, ctx_b)):
                        # custom-DVE ops mishandle base_partition != 0 inputs,
                        # so land the denominator in a base-0 tile first
                        d0_sb = work.tile([64, 512], F32, tag="den0", name=f"d0_{nc.next_id()}")
                        nc.vector.tensor_copy(out=d0_sb, in_=ctx[64:128, :])
                        d_sb = work.tile([64, 512], F32, tag="den", name=f"d_{nc.next_id()}")
                        nc.vector.reciprocal_approx_fast(out=d_sb, in_=d0_sb)
                        o_sb = work.tile([64, 512], F32, tag="outt", name=f"o_{nc.next_id()}")
                        nc.vector.tensor_tensor(
                            out=o_sb, in0=ctx[0:64, :],
                            in1=d_sb,
                            op=mybir.AluOpType.mult)
                        nc.sync.dma_start(out=out_ext[h][:, qsl], in_=o_sb)

            # emission order chosen so attention on pair 0 can start as early
            # as possible while pair-1 projections fill PE gaps
            project_T(kT, wk_sb, bk_sb, 0)
            project_T(qT, wq_sb, bq_sb, 0)
            project_v()
            attention(0)
            project_T(kT, wk_sb, bk_sb, 1)
            project_T(qT, wq_sb, bq_sb, 1)
            attention(1)

    nc.compile()
    return nc


def _get_nc(apply_mask: bool) -> bass.Bass:
    if apply_mask not in _CACHE:
        _CACHE[apply_mask] = build(apply_mask)
    return _CACHE[apply_mask]


def _in_maps(x, mask, Wq, bq, Wk, bk, Wv, bv, apply_mask):
    xT_b = [np.ascontiguousarray(x[b].T).astype(np_bf16) for b in range(B)]
    maps = []
    for c in range(NCORES):
        b, hg = c // 4, c % 4
        cs = slice(hg * COLS, (hg + 1) * COLS)
        m = {
            "xT": xT_b[b],
            "wq": np.ascontiguousarray(Wq[:, cs]).astype(np_bf16),
            "wk": np.ascontiguousarray(Wk[:, cs]).astype(np_bf16),
            "wv": np.ascontiguousarray(Wv[:, cs]).astype(np_bf16),
            "bq": np.ascontiguousarray(bq[cs].reshape(2, 128).T).astype(np.float32),
            "bk": np.ascontiguousarray(bk[cs].reshape(2, 128).T).astype(np.float32),
            "bv": np.ascontiguousarray(
                np.broadcast_to(bv[cs], (128, COLS))).astype(np.float32),
        }
        if apply_mask:
            m["maskm"] = np.ascontiguousarray(
                mask[b].astype(np.float32).reshape(MC, 128).T)
        maps.append(m)
    return maps


def _ensure_ntff_hook():
    """The agent image's antenv lacks axon_hooks; synthesize it so
    run_bass_kernel_spmd(trace=True) can reach the axon NTFF profiler."""
    import sys as _sys
    import types as _types
    try:
        from antenv import axon_hooks  # noqa: F401
        return
    except ImportError:
        pass
    import antenv
    mod = _types.ModuleType("antenv.axon_hooks")
    _hook = [None]
    mod.set_axon_ntff_profile_hook = lambda h: _hook.__setitem__(0, h)
    mod.get_axon_ntff_profile_hook = lambda: _hook[0]
    _sys.modules["antenv.axon_hooks"] = mod
    antenv.axon_hooks = mod
    from trn_agent_boot.trn_boot import _ntff_profile_via_ctypes
    mod.set_axon_ntff_profile_hook(
        _ntff_profile_via_ctypes("/opt/axon/libaxon_pjrt.so"))


def run(inputs: dict, trace: bool = False):
    if trace:
        _ensure_ntff_hook()
    x = np.asarray(inputs["x"], dtype=np.float32)
    mask = np.asarray(inputs["mask"])
    apply_mask = not bool((mask == 1).all())
    nc = _get_nc(apply_mask)
    maps = _in_maps(x, mask, np.asarray(inputs["Wq"], np.float32),
                    np.asarray(inputs["bq"], np.float32),
                    np.asarray(inputs["Wk"], np.float32),
                    np.asarray(inputs["bk"], np.float32),
                    np.asarray(inputs["Wv"], np.float32),
                    np.asarray(inputs["bv"], np.float32), apply_mask)
    res = run_bass_kernel_spmd(nc, maps, core_ids=list(range(NCORES)), trace=trace)
    out = np.empty((B, S, HID), dtype=np.float32)
    for c in range(NCORES):
        b, hg = c // 4, c % 4
        cs = slice(hg * COLS, (hg + 1) * COLS)
        ctxT = res.results[c]["out"]          # [HPC, D, S]
        out[b, :, cs] = ctxT.transpose(2, 0, 1).reshape(S, COLS)
    return out, res


def kernel(**inputs) -> np.ndarray:
    out, _ = run(inputs)
    return out


# revision 24
# speedup vs baseline: 1.0302x; 1.0302x over previous
"""Multi-head attention (B=2, S=2048, H=16, D=64) on 8 TRN2 NeuronCores.

Sharding: data parallel on batch (2) x tensor parallel on heads (16 -> 4 per
core).  Core c handles batch c//4 and heads [4*(c%4), 4*(c%4)+4).  Each core
projects q/k/v for its head group from its batch's activations, runs the
full S x S attention for its 4 heads, and writes ctx in [head, D, S] layout.
The host transposes/concatenates shards (not part of HW exec time).

Device kernel (per core, identical SPMD program, no collectives):
  - qT/kT computed directly in [D, S] layout (head pairs packed into 128
    partitions) so the scores matmul needs no transposes.
  - scoresT tiles [S_k=128, S_q] = kT_chunk.T @ qT; softmax denominator via a
    ones-column appended to v (one matmul stream produces ctx and denom).
  - exp on the scalar engine with the 1/sqrt(D) scale folded in; bf16
    matmul operands, f32 PSUM accumulation; final normalize = DVE divide.
  - padding mask folded into v_aug row zeroing (exp(x-1e4) underflows to 0
    in f32, so zeroing masked key rows is exactly equivalent).
"""

import numpy as np
import ml_dtypes

import concourse.bass as bass
import concourse.tile as tile
from concourse import bacc, mybir
from concourse.bass_utils import run_bass_kernel_spmd

B, S, H, D = 2, 2048, 16, 64
HID = H * D
NCORES = 8
HPC = 4               # heads per core
COLS = HPC * D        # 256 projection columns per core
KC = HID // 128       # 8 contraction chunks for projections
QC = S // 512         # 4 query chunks of 512
MC = S // 128         # 16 key chunks of 128

BF16 = mybir.dt.bfloat16
F32 = mybir.dt.float32
np_bf16 = ml_dtypes.bfloat16

_CACHE = {}


def build(apply_mask: bool) -> bass.Bass:
    nc = bacc.Bacc(None, target_bir_lowering=False, debug=False)

    xT = nc.declare_dram_parameter("xT", [HID, S], BF16, isOutput=False)
    wq = nc.declare_dram_parameter("wq", [HID, COLS], BF16, isOutput=False)
    wk = nc.declare_dram_parameter("wk", [HID, COLS], BF16, isOutput=False)
    wv = nc.declare_dram_parameter("wv", [HID, COLS], BF16, isOutput=False)
    bq = nc.declare_dram_parameter("bq", [128, 2], F32, isOutput=False)
    bk = nc.declare_dram_parameter("bk", [128, 2], F32, isOutput=False)
    bv = nc.declare_dram_parameter("bv", [128, COLS], F32, isOutput=False)
    if apply_mask:
        mm_in = nc.declare_dram_parameter("maskm", [128, MC], F32, isOutput=False)
    out_ext = nc.declare_dram_parameter("out", [HPC, D, S], F32, isOutput=True)

    with tile.TileContext(nc) as tc:
        with (
            tc.tile_pool(name="singles", bufs=1) as singles,
            tc.tile_pool(name="work", bufs=4) as work,
            tc.tile_pool(name="psum", bufs=2, space="PSUM") as psum,
        ):
            # ---- input DMA ----
            x_sb = singles.tile([128, KC, S], BF16)
            for kc in range(KC):
                nc.sync.dma_start(out=x_sb[:, kc, :], in_=xT[kc * 128:(kc + 1) * 128, :])

            wq_sb = singles.tile([128, KC, COLS], BF16)
            wk_sb = singles.tile([128, KC, COLS], BF16)
            wv_sb = singles.tile([128, KC, COLS], BF16)
            for w_sb, w_in in ((wq_sb, wq), (wk_sb, wk), (wv_sb, wv)):
                for kc in range(KC):
                    nc.sync.dma_start(out=w_sb[:, kc, :], in_=w_in[kc * 128:(kc + 1) * 128, :])

            bq_sb = singles.tile([128, 2], F32)
            nc.sync.dma_start(out=bq_sb, in_=bq[:, :])
            bk_sb = singles.tile([128, 2], F32)
            nc.sync.dma_start(out=bk_sb, in_=bk[:, :])
            bv_sb = singles.tile([128, COLS], F32)
            nc.sync.dma_start(out=bv_sb, in_=bv[:, :])
            if apply_mask:
                mm_sb = singles.tile([128, MC], F32)
                nc.sync.dma_start(out=mm_sb, in_=mm_in[:, :])

            # ---- projections ----
            # qT/kT: [128, pair, S]; partitions 0:64 = head 2p, 64:128 = head 2p+1
            qT = singles.tile([128, 2, S], BF16)
            kT = singles.tile([128, 2, S], BF16)
            # v_aug: [128, key_chunk, head, 128]; cols 64:128 are ones columns,
            # so the ctx matmul emits the softmax denominator replicated into
            # psum partitions 64:128 at no extra cost (matmul cost is N-bound)
            v_aug = singles.tile([128, MC, HPC, 128], BF16)
            nc.vector.memset(v_aug[:, :, :, 64:128], 1.0)

            def project_T(dst, w_sb, b_sb, p):
                for qc in range(QC):
                    ps = psum.tile([128, 512], F32, tag="proj_ps", name=f"pt_{nc.next_id()}")
                    for kc in range(KC):
                        nc.tensor.matmul(
                            ps,
                            lhsT=w_sb[:, kc, p * 128:(p + 1) * 128],
                            rhs=x_sb[:, kc, qc * 512:(qc + 1) * 512],
                            start=(kc == 0), stop=(kc == KC - 1),
                        )
                    nc.vector.tensor_tensor(
                        out=dst[:, p, qc * 512:(qc + 1) * 512],
                        in0=ps,
                        in1=b_sb[:, p:p + 1].to_broadcast([128, 512]),
                        op=mybir.AluOpType.add,
                    )

            def project_v_chunk(mc):
                ps = psum.tile([128, COLS], F32, tag="proj_ps", name=f"pv_{nc.next_id()}")
                for kc in range(KC):
                    nc.tensor.matmul(
                        ps,
                        lhsT=x_sb[:, kc, mc * 128:(mc + 1) * 128],
                        rhs=wv_sb[:, kc, :],
                        start=(kc == 0), stop=(kc == KC - 1),
                    )
                nc.vector.tensor_tensor(
                    out=v_aug[:, mc, :, 0:64],
                    in0=ps[:, :].rearrange("p (h d) -> p h d", h=HPC),
                    in1=bv_sb.rearrange("p (h d) -> p h d", h=HPC),
                    op=mybir.AluOpType.add,
                )
                if apply_mask:
                    nc.vector.tensor_tensor(
                        out=v_aug[:, mc, :, :],
                        in0=v_aug[:, mc, :, :],
                        in1=mm_sb[:, mc:mc + 1, None].to_broadcast([128, HPC, 128]),
                        op=mybir.AluOpType.mult,
                    )

            def attention(p, emit_v=False):
                ha, hb = 2 * p, 2 * p + 1
                for qc in range(QC):
                    qsl = slice(qc * 512, (qc + 1) * 512)
                    ctx_a = psum.tile([128, 512], F32, tag="ctx", name=f"ca_{nc.next_id()}")
                    ctx_b = psum.tile([128, 512], F32, tag="ctx", name=f"cb_{nc.next_id()}")
                    for kc2 in range(MC // 2):
                        kc0, kc1 = 2 * kc2, 2 * kc2 + 1
                        if emit_v and qc == 0:
                            project_v_chunk(kc0)
                            project_v_chunk(kc1)
                        s_a = psum.tile([128, 1024], F32, tag="sps", name=f"sa_{nc.next_id()}")
                        s_b = psum.tile([128, 1024], F32, tag="sps", name=f"sb_{nc.next_id()}")
                        # paired row-group matmuls: head a on array rows 0:63,
                        # head b on rows 64:127 run concurrently
                        for i, kc in enumerate((kc0, kc1)):
                            ksl = slice(kc * 128, (kc + 1) * 128)
                            csl = slice(i * 512, (i + 1) * 512)
                            nc.tensor.matmul(
                                s_a[:, csl], lhsT=kT[0:64, p, ksl], rhs=qT[0:64, p, qsl],
                                start=True, stop=True)
                            nc.tensor.matmul(
                                s_b[:, csl], lhsT=kT[64:128, p, ksl], rhs=qT[64:128, p, qsl],
                                start=True, stop=True)
                        e_a = work.tile([128, 1024], BF16, tag="expT", name=f"ea_{nc.next_id()}")
                        e_b = work.tile([128, 1024], BF16, tag="expT", name=f"eb_{nc.next_id()}")
                        nc.scalar.activation(e_a, s_a, mybir.ActivationFunctionType.Exp,
                                             scale=0.125)
                        nc.scalar.activation(e_b, s_b, mybir.ActivationFunctionType.Exp,
                                             scale=0.125)
                        for i, kc in enumerate((kc0, kc1)):
                            csl = slice(i * 512, (i + 1) * 512)
                            nc.tensor.matmul(
                                ctx_a, lhsT=v_aug[:, kc, ha, :], rhs=e_a[:, csl],
                                start=(kc == 0), stop=(kc == MC - 1))
                            nc.tensor.matmul(
                                ctx_b, lhsT=v_aug[:, kc, hb, :], rhs=e_b[:, csl],
                                start=(kc == 0), stop=(kc == MC - 1))
                    for h, ctx in ((ha, ctx_a), (hb, ctx_b)):
                        # custom-DVE ops mishandle base_partition != 0 inputs,
                        # so land the denominator in a base-0 tile first
                        d0_sb = work.tile([64, 512], F32, tag="den0", name=f"d0_{nc.next_id()}")
                        nc.vector.tensor_copy(out=d0_sb, in_=ctx[64:128, :])
                        d_sb = work.tile([64, 512], F32, tag="den", name=f"d_{nc.next_id()}")
                        nc.vector.reciprocal_approx_fast(out=d_sb, in_=d0_sb)
                        o_sb = work.tile([64, 512], F32, tag="outt", name=f"o_{nc.next_id()}")
                        nc.vector.tensor_tensor(
                            out=o_sb, in0=ctx[0:64, :],
                            in1=d_sb,
                            op=mybir.AluOpType.mult)
                        nc.sync.dma_start(out=out_ext[h][:, qsl], in_=o_sb)

            # emission order chosen so attention on pair 0 can start as early
            # as possible; v / pair-1 projections fill PE gaps while the
            # scalar engine grinds pair-0 exps
            project_T(kT, wk_sb, bk_sb, 0)
            project_T(qT, wq_sb, bq_sb, 0)
            attention(0, emit_v=True)
            project_T(kT, wk_sb, bk_sb, 1)
            project_T(qT, wq_sb, bq_sb, 1)
            attention(1)

    nc.compile()
    return nc


def _get_nc(apply_mask: bool) -> bass.Bass:
    if apply_mask not in _CACHE:
        _CACHE[apply_mask] = build(apply_mask)
    return _CACHE[apply_mask]


def _in_maps(x, mask, Wq, bq, Wk, bk, Wv, bv, apply_mask):
    xT_b = [np.ascontiguousarray(x[b].T).astype(np_bf16) for b in range(B)]
    maps = []
    for c in range(NCORES):
        b, hg = c // 4, c % 4
        cs = slice(hg * COLS, (hg + 1) * COLS)
        m = {
            "xT": xT_b[b],
            "wq": np.ascontiguousarray(Wq[:, cs]).astype(np_bf16),
            "wk": np.ascontiguousarray(Wk[:, cs]).astype(np_bf16),
            "wv": np.ascontiguousarray(Wv[:, cs]).astype(np_bf16),
            "bq": np.ascontiguousarray(bq[cs].reshape(2, 128).T).astype(np.float32),
            "bk": np.ascontiguousarray(bk[cs].reshape(2, 128).T).astype(np.float32),
            "bv": np.ascontiguousarray(
                np.broadcast_to(bv[cs], (128, COLS))).astype(np.float32),
        }
        if apply_mask:
            m["maskm"] = np.ascontiguousarray(
                mask[b].astype(np.float32).reshape(MC, 128).T)
        maps.append(m)
    return maps


def _ensure_ntff_hook():
    """The agent image's antenv lacks axon_hooks; synthesize it so
    run_bass_kernel_spmd(trace=True) can reach the axon NTFF profiler."""
    import sys as _sys
    import types as _types
    try:
        from antenv import axon_hooks  # noqa: F401
        return
    except ImportError:
        pass
    import antenv
    mod = _types.ModuleType("antenv.axon_hooks")
    _hook = [None]
    mod.set_axon_ntff_profile_hook = lambda h: _hook.__setitem__(0, h)
    mod.get_axon_ntff_profile_hook = lambda: _hook[0]
    _sys.modules["antenv.axon_hooks"] = mod
    antenv.axon_hooks = mod
    from trn_agent_boot.trn_boot import _ntff_profile_via_ctypes
    mod.set_axon_ntff_profile_hook(
        _ntff_profile_via_ctypes("/opt/axon/libaxon_pjrt.so"))


def run(inputs: dict, trace: bool = False):
    if trace:
        _ensure_ntff_hook()
    x = np.asarray(inputs["x"], dtype=np.float32)
    mask = np.asarray(inputs["mask"])
    apply_mask = not bool((mask == 1).all())
    nc = _get_nc(apply_mask)
    maps = _in_maps(x, mask, np.asarray(inputs["Wq"], np.float32),
                    np.asarray(inputs["bq"], np.float32),
                    np.asarray(inputs["Wk"], np.float32),
                    np.asarray(inputs["bk"], np.float32),
                    np.asarray(inputs["Wv"], np.float32),
                    np.asarray(inputs["bv"], np.float32), apply_mask)
    res = run_bass_kernel_spmd(nc, maps, core_ids=list(range(NCORES)), trace=trace)
    out = np.empty((B, S, HID), dtype=np.float32)
    for c in range(NCORES):
        b, hg = c // 4, c % 4
        cs = slice(hg * COLS, (hg + 1) * COLS)
        ctxT = res.results[c]["out"]          # [HPC, D, S]
        out[b, :, cs] = ctxT.transpose(2, 0, 1).reshape(S, COLS)
    return out, res


def kernel(**inputs) -> np.ndarray:
    out, _ = run(inputs)
    return out


# revision 25
# speedup vs baseline: 1.0322x; 1.0019x over previous
"""Multi-head attention (B=2, S=2048, H=16, D=64) on 8 TRN2 NeuronCores.

Sharding: data parallel on batch (2) x tensor parallel on heads (16 -> 4 per
core).  Core c handles batch c//4 and heads [4*(c%4), 4*(c%4)+4).  Each core
projects q/k/v for its head group from its batch's activations, runs the
full S x S attention for its 4 heads, and writes ctx in [head, D, S] layout.
The host transposes/concatenates shards (not part of HW exec time).

Device kernel (per core, identical SPMD program, no collectives):
  - qT/kT computed directly in [D, S] layout (head pairs packed into 128
    partitions) so the scores matmul needs no transposes.
  - scoresT tiles [S_k=128, S_q] = kT_chunk.T @ qT; softmax denominator via a
    ones-column appended to v (one matmul stream produces ctx and denom).
  - exp on the scalar engine with the 1/sqrt(D) scale folded in; bf16
    matmul operands, f32 PSUM accumulation; final normalize = DVE divide.
  - padding mask folded into v_aug row zeroing (exp(x-1e4) underflows to 0
    in f32, so zeroing masked key rows is exactly equivalent).
"""

import numpy as np
import ml_dtypes

import concourse.bass as bass
import concourse.tile as tile
from concourse import bacc, mybir
from concourse.bass_utils import run_bass_kernel_spmd

B, S, H, D = 2, 2048, 16, 64
HID = H * D
NCORES = 8
HPC = 4               # heads per core
COLS = HPC * D        # 256 projection columns per core
KC = HID // 128       # 8 contraction chunks for projections
QC = S // 512         # 4 query chunks of 512
MC = S // 128         # 16 key chunks of 128

BF16 = mybir.dt.bfloat16
F32 = mybir.dt.float32
np_bf16 = ml_dtypes.bfloat16

_CACHE = {}


def build(apply_mask: bool) -> bass.Bass:
    nc = bacc.Bacc(None, target_bir_lowering=False, debug=False)

    xT = nc.declare_dram_parameter("xT", [HID, S], BF16, isOutput=False)
    wq = nc.declare_dram_parameter("wq", [HID, COLS], BF16, isOutput=False)
    wk = nc.declare_dram_parameter("wk", [HID, COLS], BF16, isOutput=False)
    wv = nc.declare_dram_parameter("wv", [HID, COLS], BF16, isOutput=False)
    bq = nc.declare_dram_parameter("bq", [128, 2], F32, isOutput=False)
    bk = nc.declare_dram_parameter("bk", [128, 2], F32, isOutput=False)
    bv = nc.declare_dram_parameter("bv", [128, COLS], F32, isOutput=False)
    if apply_mask:
        mm_in = nc.declare_dram_parameter("maskm", [128, MC], F32, isOutput=False)
    out_ext = nc.declare_dram_parameter("out", [HPC, D, S], F32, isOutput=True)

    with tile.TileContext(nc) as tc:
        with (
            tc.tile_pool(name="singles", bufs=1) as singles,
            tc.tile_pool(name="work", bufs=4) as work,
            tc.tile_pool(name="psum", bufs=2, space="PSUM") as psum,
        ):
            # ---- input DMA ----
            x_sb = singles.tile([128, KC, S], BF16)
            for kc in range(KC):
                nc.sync.dma_start(out=x_sb[:, kc, :], in_=xT[kc * 128:(kc + 1) * 128, :])

            wq_sb = singles.tile([128, KC, COLS], BF16)
            wk_sb = singles.tile([128, KC, COLS], BF16)
            wv_sb = singles.tile([128, KC, COLS], BF16)
            for w_sb, w_in in ((wq_sb, wq), (wk_sb, wk), (wv_sb, wv)):
                for kc in range(KC):
                    nc.sync.dma_start(out=w_sb[:, kc, :], in_=w_in[kc * 128:(kc + 1) * 128, :])

            bq_sb = singles.tile([128, 2], F32)
            nc.sync.dma_start(out=bq_sb, in_=bq[:, :])
            bk_sb = singles.tile([128, 2], F32)
            nc.sync.dma_start(out=bk_sb, in_=bk[:, :])
            bv_sb = singles.tile([128, COLS], F32)
            nc.sync.dma_start(out=bv_sb, in_=bv[:, :])
            if apply_mask:
                mm_sb = singles.tile([128, MC], F32)
                nc.sync.dma_start(out=mm_sb, in_=mm_in[:, :])

            # ---- projections ----
            # qT/kT: [128, pair, S]; partitions 0:64 = head 2p, 64:128 = head 2p+1
            qT = singles.tile([128, 2, S], BF16)
            kT = singles.tile([128, 2, S], BF16)
            # v_aug: [128, key_chunk, head, 128]; cols 64:128 are ones columns,
            # so the ctx matmul emits the softmax denominator replicated into
            # psum partitions 64:128 at no extra cost (matmul cost is N-bound)
            v_aug = singles.tile([128, MC, HPC, 128], BF16)
            nc.vector.memset(v_aug[:, :, :, 64:128], 1.0)

            def project_T(dst, w_sb, b_sb, p):
                for qc in range(QC):
                    ps = psum.tile([128, 512], F32, tag="proj_ps", name=f"pt_{nc.next_id()}")
                    for kc in range(KC):
                        nc.tensor.matmul(
                            ps,
                            lhsT=w_sb[:, kc, p * 128:(p + 1) * 128],
                            rhs=x_sb[:, kc, qc * 512:(qc + 1) * 512],
                            start=(kc == 0), stop=(kc == KC - 1),
                        )
                    nc.vector.tensor_tensor(
                        out=dst[:, p, qc * 512:(qc + 1) * 512],
                        in0=ps,
                        in1=b_sb[:, p:p + 1].to_broadcast([128, 512]),
                        op=mybir.AluOpType.add,
                    )

            def project_v_chunk(mc):
                ps = psum.tile([128, COLS], F32, tag="proj_ps", name=f"pv_{nc.next_id()}")
                for kc in range(KC):
                    nc.tensor.matmul(
                        ps,
                        lhsT=x_sb[:, kc, mc * 128:(mc + 1) * 128],
                        rhs=wv_sb[:, kc, :],
                        start=(kc == 0), stop=(kc == KC - 1),
                    )
                nc.vector.tensor_tensor(
                    out=v_aug[:, mc, :, 0:64],
                    in0=ps[:, :].rearrange("p (h d) -> p h d", h=HPC),
                    in1=bv_sb.rearrange("p (h d) -> p h d", h=HPC),
                    op=mybir.AluOpType.add,
                )
                if apply_mask:
                    nc.vector.tensor_tensor(
                        out=v_aug[:, mc, :, :],
                        in0=v_aug[:, mc, :, :],
                        in1=mm_sb[:, mc:mc + 1, None].to_broadcast([128, HPC, 128]),
                        op=mybir.AluOpType.mult,
                    )

            def attention(p, emit_v=False):
                ha, hb = 2 * p, 2 * p + 1
                for qc in range(QC):
                    qsl = slice(qc * 512, (qc + 1) * 512)
                    ctx_a = psum.tile([128, 512], F32, tag="ctx", name=f"ca_{nc.next_id()}")
                    ctx_b = psum.tile([128, 512], F32, tag="ctx", name=f"cb_{nc.next_id()}")
                    for kc2 in range(MC // 2):
                        kc0, kc1 = 2 * kc2, 2 * kc2 + 1
                        if emit_v and qc == 0:
                            project_v_chunk(kc0)
                            project_v_chunk(kc1)
                        s_a = psum.tile([128, 1024], F32, tag="sps", name=f"sa_{nc.next_id()}")
                        s_b = psum.tile([128, 1024], F32, tag="sps", name=f"sb_{nc.next_id()}")
                        # paired row-group matmuls: head a on array rows 0:63,
                        # head b on rows 64:127 run concurrently
                        for i, kc in enumerate((kc0, kc1)):
                            ksl = slice(kc * 128, (kc + 1) * 128)
                            csl = slice(i * 512, (i + 1) * 512)
                            nc.tensor.matmul(
                                s_a[:, csl], lhsT=kT[0:64, p, ksl], rhs=qT[0:64, p, qsl],
                                start=True, stop=True)
                            nc.tensor.matmul(
                                s_b[:, csl], lhsT=kT[64:128, p, ksl], rhs=qT[64:128, p, qsl],
                                start=True, stop=True)
                        e_a = work.tile([128, 1024], BF16, tag="expT", name=f"ea_{nc.next_id()}")
                        e_b = work.tile([128, 1024], BF16, tag="expT", name=f"eb_{nc.next_id()}")
                        nc.scalar.activation(e_a, s_a, mybir.ActivationFunctionType.Exp,
                                             scale=0.125)
                        nc.scalar.activation(e_b, s_b, mybir.ActivationFunctionType.Exp,
                                             scale=0.125)
                        for i, kc in enumerate((kc0, kc1)):
                            csl = slice(i * 512, (i + 1) * 512)
                            nc.tensor.matmul(
                                ctx_a, lhsT=v_aug[:, kc, ha, :], rhs=e_a[:, csl],
                                start=(kc == 0), stop=(kc == MC - 1))
                            nc.tensor.matmul(
                                ctx_b, lhsT=v_aug[:, kc, hb, :], rhs=e_b[:, csl],
                                start=(kc == 0), stop=(kc == MC - 1))
                    for h, ctx in ((ha, ctx_a), (hb, ctx_b)):
                        # one copy releases the ctx psum bank immediately;
                        # custom-DVE recip needs a base-0 input tile
                        g_sb = work.tile([128, 512], F32, tag="gctx", name=f"g_{nc.next_id()}")
                        nc.vector.tensor_copy(out=g_sb, in_=ctx)
                        d0_sb = work.tile([64, 512], F32, tag="den0", name=f"d0_{nc.next_id()}")
                        nc.vector.tensor_copy(out=d0_sb, in_=g_sb[64:128, :])
                        d_sb = work.tile([64, 512], F32, tag="den", name=f"d_{nc.next_id()}")
                        nc.vector.reciprocal_approx_fast(out=d_sb, in_=d0_sb)
                        o_sb = work.tile([64, 512], F32, tag="outt", name=f"o_{nc.next_id()}")
                        nc.vector.tensor_tensor(
                            out=o_sb, in0=g_sb[0:64, :],
                            in1=d_sb,
                            op=mybir.AluOpType.mult)
                        nc.sync.dma_start(out=out_ext[h][:, qsl], in_=o_sb)

            # emission order chosen so attention on pair 0 can start as early
            # as possible; v / pair-1 projections fill PE gaps while the
            # scalar engine grinds pair-0 exps
            project_T(kT, wk_sb, bk_sb, 0)
            project_T(qT, wq_sb, bq_sb, 0)
            attention(0, emit_v=True)
            project_T(kT, wk_sb, bk_sb, 1)
            project_T(qT, wq_sb, bq_sb, 1)
            attention(1)

    nc.compile()
    return nc


def _get_nc(apply_mask: bool) -> bass.Bass:
    if apply_mask not in _CACHE:
        _CACHE[apply_mask] = build(apply_mask)
    return _CACHE[apply_mask]


def _in_maps(x, mask, Wq, bq, Wk, bk, Wv, bv, apply_mask):
    xT_b = [np.ascontiguousarray(x[b].T).astype(np_bf16) for b in range(B)]
    maps = []
    for c in range(NCORES):
        b, hg = c // 4, c % 4
        cs = slice(hg * COLS, (hg + 1) * COLS)
        m = {
            "xT": xT_b[b],
            "wq": np.ascontiguousarray(Wq[:, cs]).astype(np_bf16),
            "wk": np.ascontiguousarray(Wk[:, cs]).astype(np_bf16),
            "wv": np.ascontiguousarray(Wv[:, cs]).astype(np_bf16),
            "bq": np.ascontiguousarray(bq[cs].reshape(2, 128).T).astype(np.float32),
            "bk": np.ascontiguousarray(bk[cs].reshape(2, 128).T).astype(np.float32),
            "bv": np.ascontiguousarray(
                np.broadcast_to(bv[cs], (128, COLS))).astype(np.float32),
        }
        if apply_mask:
            m["maskm"] = np.ascontiguousarray(
                mask[b].astype(np.float32).reshape(MC, 128).T)
        maps.append(m)
    return maps


def _ensure_ntff_hook():
    """The agent image's antenv lacks axon_hooks; synthesize it so
    run_bass_kernel_spmd(trace=True) can reach the axon NTFF profiler."""
    import sys as _sys
    import types as _types
    try:
        from antenv import axon_hooks  # noqa: F401
        return
    except ImportError:
        pass
    import antenv
    mod = _types.ModuleType("antenv.axon_hooks")
    _hook = [None]
    mod.set_axon_ntff_profile_hook = lambda h: _hook.__setitem__(0, h)
    mod.get_axon_ntff_profile_hook = lambda: _hook[0]
    _sys.modules["antenv.axon_hooks"] = mod
    antenv.axon_hooks = mod
    from trn_agent_boot.trn_boot import _ntff_profile_via_ctypes
    mod.set_axon_ntff_profile_hook(
        _ntff_profile_via_ctypes("/opt/axon/libaxon_pjrt.so"))


def run(inputs: dict, trace: bool = False):
    if trace:
        _ensure_ntff_hook()
    x = np.asarray(inputs["x"], dtype=np.float32)
    mask = np.asarray(inputs["mask"])
    apply_mask = not bool((mask == 1).all())
    nc = _get_nc(apply_mask)
    maps = _in_maps(x, mask, np.asarray(inputs["Wq"], np.float32),
                    np.asarray(inputs["bq"], np.float32),
                    np.asarray(inputs["Wk"], np.float32),
                    np.asarray(inputs["bk"], np.float32),
                    np.asarray(inputs["Wv"], np.float32),
                    np.asarray(inputs["bv"], np.float32), apply_mask)
    res = run_bass_kernel_spmd(nc, maps, core_ids=list(range(NCORES)), trace=trace)
    out = np.empty((B, S, HID), dtype=np.float32)
    for c in range(NCORES):
        b, hg = c // 4, c % 4
        cs = slice(hg * COLS, (hg + 1) * COLS)
        ctxT = res.results[c]["out"]          # [HPC, D, S]
        out[b, :, cs] = ctxT.transpose(2, 0, 1).reshape(S, COLS)
    return out, res


def kernel(**inputs) -> np.ndarray:
    out, _ = run(inputs)
    return out


# revision 29
# speedup vs baseline: 1.0831x; 1.0493x over previous
"""Multi-head attention (B=2, S=2048, H=16, D=64) on 8 TRN2 NeuronCores.

Sharding: data parallel on batch (2) x tensor parallel on heads (16 -> 4 per
core).  Core c handles batch c//4 and heads [4*(c%4), 4*(c%4)+4).  Each core
projects q/k/v for its head group from its batch's activations, runs the
full S x S attention for its 4 heads, and writes ctx in [head, D, S] layout.
The host transposes/concatenates shards (not part of HW exec time).

Device kernel (per core, identical SPMD program, no collectives):
  - qT/kT computed directly in [D, S] layout (head pairs packed into 128
    partitions) so the scores matmul needs no transposes.
  - scoresT tiles [S_k=128, S_q] = kT_chunk.T @ qT; softmax denominator via a
    ones-column appended to v (one matmul stream produces ctx and denom).
  - exp on the scalar engine with the 1/sqrt(D) scale folded in; bf16
    matmul operands, f32 PSUM accumulation; final normalize = DVE divide.
  - padding mask folded into v_aug row zeroing (exp(x-1e4) underflows to 0
    in f32, so zeroing masked key rows is exactly equivalent).
"""

import numpy as np
import ml_dtypes

import concourse.bass as bass
import concourse.tile as tile
from concourse import bacc, mybir
from concourse.bass_utils import run_bass_kernel_spmd

B, S, H, D = 2, 2048, 16, 64
HID = H * D
NCORES = 8
HPC = 4               # heads per core
COLS = HPC * D        # 256 projection columns per core
KC = HID // 128       # 8 contraction chunks for projections
QC = S // 512         # 4 query chunks of 512
MC = S // 128         # 16 key chunks of 128

BF16 = mybir.dt.bfloat16
F32 = mybir.dt.float32
np_bf16 = ml_dtypes.bfloat16

_CACHE = {}


def build(apply_mask: bool) -> bass.Bass:
    nc = bacc.Bacc(None, target_bir_lowering=False, debug=False)

    xT = nc.declare_dram_parameter("xT", [HID, S], BF16, isOutput=False)
    wq = nc.declare_dram_parameter("wq", [HID, COLS], BF16, isOutput=False)
    wk = nc.declare_dram_parameter("wk", [HID, COLS], BF16, isOutput=False)
    wv = nc.declare_dram_parameter("wv", [HID, COLS], BF16, isOutput=False)
    bq = nc.declare_dram_parameter("bq", [128, 2], F32, isOutput=False)
    bk = nc.declare_dram_parameter("bk", [128, 2], F32, isOutput=False)
    bv = nc.declare_dram_parameter("bv", [128, COLS], F32, isOutput=False)
    if apply_mask:
        mm_in = nc.declare_dram_parameter("maskm", [128, MC], F32, isOutput=False)
    out_ext = nc.declare_dram_parameter("out", [HPC, D, S], F32, isOutput=True)

    with tile.TileContext(nc) as tc:
        with (
            tc.tile_pool(name="singles", bufs=1) as singles,
            tc.tile_pool(name="work", bufs=4) as work,
            tc.tile_pool(name="psum", bufs=2, space="PSUM") as psum,
        ):
            # ---- input DMA ----
            x_sb = singles.tile([128, KC, S], BF16)
            for kc in range(KC):
                nc.sync.dma_start(out=x_sb[:, kc, :], in_=xT[kc * 128:(kc + 1) * 128, :])

            wq_sb = singles.tile([128, KC, COLS], BF16)
            wk_sb = singles.tile([128, KC, COLS], BF16)
            wv_sb = singles.tile([128, KC, COLS], BF16)
            for w_sb, w_in in ((wq_sb, wq), (wk_sb, wk), (wv_sb, wv)):
                for kc in range(KC):
                    nc.sync.dma_start(out=w_sb[:, kc, :], in_=w_in[kc * 128:(kc + 1) * 128, :])

            bq_sb = singles.tile([128, 2], F32)
            nc.sync.dma_start(out=bq_sb, in_=bq[:, :])
            bk_sb = singles.tile([128, 2], F32)
            nc.sync.dma_start(out=bk_sb, in_=bk[:, :])
            bv_sb = singles.tile([128, COLS], F32)
            nc.sync.dma_start(out=bv_sb, in_=bv[:, :])
            if apply_mask:
                mm_sb = singles.tile([128, MC], F32)
                nc.sync.dma_start(out=mm_sb, in_=mm_in[:, :])

            # ---- projections ----
            # kT: [128, pair, S]; partitions 0:64 = head 2p, 64:128 = head 2p+1
            # qTz: zero-padded per head so score matmuls run full-row K=128
            # (variant 0: head-a rows live, b rows zero; variant 1 reversed)
            qTz = singles.tile([128, 2, 2, S], BF16)
            nc.vector.memset(qTz[64:128, :, 0, :], 0.0)
            nc.vector.memset(qTz[0:64, :, 1, :], 0.0)
            kT = singles.tile([128, 2, S], BF16)
            # v_aug: [128, key_chunk, head, 128]; cols 64:128 are ones columns,
            # so the ctx matmul emits the softmax denominator replicated into
            # psum partitions 64:128 at no extra cost (matmul cost is N-bound)
            v_aug = singles.tile([128, MC, HPC, 128], BF16)
            nc.vector.memset(v_aug[:, :, :, 64:128], 1.0)

            def project_T(dst, w_sb, b_sb, p, zpad=False):
                for qc in range(QC):
                    ps = psum.tile([128, 512], F32, tag="proj_ps", name=f"pt_{nc.next_id()}")
                    for kc in range(KC):
                        nc.tensor.matmul(
                            ps,
                            lhsT=w_sb[:, kc, p * 128:(p + 1) * 128],
                            rhs=x_sb[:, kc, qc * 512:(qc + 1) * 512],
                            start=(kc == 0), stop=(kc == KC - 1),
                        )
                    qsl = slice(qc * 512, (qc + 1) * 512)
                    if zpad:
                        nc.vector.tensor_tensor(
                            out=dst[0:64, p, 0, qsl],
                            in0=ps[0:64, :],
                            in1=b_sb[0:64, p:p + 1].to_broadcast([64, 512]),
                            op=mybir.AluOpType.add,
                        )
                        nc.vector.tensor_tensor(
                            out=dst[64:128, p, 1, qsl],
                            in0=ps[64:128, :],
                            in1=b_sb[64:128, p:p + 1].to_broadcast([64, 512]),
                            op=mybir.AluOpType.add,
                        )
                    else:
                        nc.vector.tensor_tensor(
                            out=dst[:, p, qsl],
                            in0=ps,
                            in1=b_sb[:, p:p + 1].to_broadcast([128, 512]),
                            op=mybir.AluOpType.add,
                        )

            def project_v_chunk(mc):
                ps = psum.tile([128, COLS], F32, tag="proj_ps", name=f"pv_{nc.next_id()}")
                for kc in range(KC):
                    nc.tensor.matmul(
                        ps,
                        lhsT=x_sb[:, kc, mc * 128:(mc + 1) * 128],
                        rhs=wv_sb[:, kc, :],
                        start=(kc == 0), stop=(kc == KC - 1),
                    )
                nc.vector.tensor_tensor(
                    out=v_aug[:, mc, :, 0:64],
                    in0=ps[:, :].rearrange("p (h d) -> p h d", h=HPC),
                    in1=bv_sb.rearrange("p (h d) -> p h d", h=HPC),
                    op=mybir.AluOpType.add,
                )
                if apply_mask:
                    nc.vector.tensor_tensor(
                        out=v_aug[:, mc, :, :],
                        in0=v_aug[:, mc, :, :],
                        in1=mm_sb[:, mc:mc + 1, None].to_broadcast([128, HPC, 128]),
                        op=mybir.AluOpType.mult,
                    )

            def attention(p, emit_v=False):
                ha, hb = 2 * p, 2 * p + 1
                for qc in range(QC):
                    qsl = slice(qc * 512, (qc + 1) * 512)
                    ctx_a = psum.tile([128, 512], F32, tag="ctx", name=f"ca_{nc.next_id()}")
                    ctx_b = psum.tile([128, 512], F32, tag="ctx", name=f"cb_{nc.next_id()}")
                    for kc2 in range(MC // 2):
                        kc0, kc1 = 2 * kc2, 2 * kc2 + 1
                        if emit_v and qc == 0:
                            project_v_chunk(kc0)
                            project_v_chunk(kc1)
                        s_a = psum.tile([128, 1024], F32, tag="sps", name=f"sa_{nc.next_id()}")
                        s_b = psum.tile([128, 1024], F32, tag="sps", name=f"sb_{nc.next_id()}")
                        # paired row-group matmuls: head a on array rows 0:63,
                        # head b on rows 64:127 run concurrently
                        for i, kc in enumerate((kc0, kc1)):
                            ksl = slice(kc * 128, (kc + 1) * 128)
                            csl = slice(i * 512, (i + 1) * 512)
                            nc.tensor.matmul(
                                s_a[:, csl], lhsT=kT[:, p, ksl], rhs=qTz[:, p, 0, qsl],
                                start=True, stop=True)
                            nc.tensor.matmul(
                                s_b[:, csl], lhsT=kT[:, p, ksl], rhs=qTz[:, p, 1, qsl],
                                start=True, stop=True)
                        e_a = work.tile([128, 1024], BF16, tag="expT", name=f"ea_{nc.next_id()}")
                        e_b = work.tile([128, 1024], BF16, tag="expT", name=f"eb_{nc.next_id()}")
                        nc.scalar.activation(e_a, s_a, mybir.ActivationFunctionType.Exp,
                                             scale=0.125)
                        nc.scalar.activation(e_b, s_b, mybir.ActivationFunctionType.Exp,
                                             scale=0.125)
                        for i, kc in enumerate((kc0, kc1)):
                            csl = slice(i * 512, (i + 1) * 512)
                            nc.tensor.matmul(
                                ctx_a, lhsT=v_aug[:, kc, ha, :], rhs=e_a[:, csl],
                                start=(kc == 0), stop=(kc == MC - 1))
                            nc.tensor.matmul(
                                ctx_b, lhsT=v_aug[:, kc, hb, :], rhs=e_b[:, csl],
                                start=(kc == 0), stop=(kc == MC - 1))
                    for h, ctx in ((ha, ctx_a), (hb, ctx_b)):
                        # one copy releases the ctx psum bank immediately;
                        # custom-DVE recip needs a base-0 input tile
                        g_sb = work.tile([128, 512], F32, tag="gctx", name=f"g_{nc.next_id()}")
                        nc.vector.tensor_copy(out=g_sb, in_=ctx)
                        d0_sb = work.tile([64, 512], F32, tag="den0", name=f"d0_{nc.next_id()}")
                        nc.vector.tensor_copy(out=d0_sb, in_=g_sb[64:128, :])
                        d_sb = work.tile([64, 512], F32, tag="den", name=f"d_{nc.next_id()}")
                        nc.vector.reciprocal_approx_fast(out=d_sb, in_=d0_sb)
                        o_sb = work.tile([64, 512], F32, tag="outt", name=f"o_{nc.next_id()}")
                        nc.vector.tensor_tensor(
                            out=o_sb, in0=g_sb[0:64, :],
                            in1=d_sb,
                            op=mybir.AluOpType.mult)
                        nc.sync.dma_start(out=out_ext[h][:, qsl], in_=o_sb)

            # emission order chosen so attention on pair 0 can start as early
            # as possible; v / pair-1 projections fill PE gaps while the
            # scalar engine grinds pair-0 exps
            project_T(kT, wk_sb, bk_sb, 0)
            project_T(qTz, wq_sb, bq_sb, 0, zpad=True)
            attention(0, emit_v=True)
            project_T(kT, wk_sb, bk_sb, 1)
            project_T(qTz, wq_sb, bq_sb, 1, zpad=True)
            attention(1)

    nc.compile()
    return nc


def _get_nc(apply_mask: bool) -> bass.Bass:
    if apply_mask not in _CACHE:
        _CACHE[apply_mask] = build(apply_mask)
    return _CACHE[apply_mask]


def _in_maps(x, mask, Wq, bq, Wk, bk, Wv, bv, apply_mask):
    xT_b = [np.ascontiguousarray(x[b].T).astype(np_bf16) for b in range(B)]
    maps = []
    for c in range(NCORES):
        b, hg = c // 4, c % 4
        cs = slice(hg * COLS, (hg + 1) * COLS)
        m = {
            "xT": xT_b[b],
            "wq": np.ascontiguousarray(Wq[:, cs]).astype(np_bf16),
            "wk": np.ascontiguousarray(Wk[:, cs]).astype(np_bf16),
            "wv": np.ascontiguousarray(Wv[:, cs]).astype(np_bf16),
            "bq": np.ascontiguousarray(bq[cs].reshape(2, 128).T).astype(np.float32),
            "bk": np.ascontiguousarray(bk[cs].reshape(2, 128).T).astype(np.float32),
            "bv": np.ascontiguousarray(
                np.broadcast_to(bv[cs], (128, COLS))).astype(np.float32),
        }
        if apply_mask:
            m["maskm"] = np.ascontiguousarray(
                mask[b].astype(np.float32).reshape(MC, 128).T)
        maps.append(m)
    return maps


def _ensure_ntff_hook():
    """The agent image's antenv lacks axon_hooks; synthesize it so
    run_bass_kernel_spmd(trace=True) can reach the axon NTFF profiler."""
    import sys as _sys
    import types as _types
    try:
        from antenv import axon_hooks  # noqa: F401
        return
    except ImportError:
        pass
    import antenv
    mod = _types.ModuleType("antenv.axon_hooks")
    _hook = [None]
    mod.set_axon_ntff_profile_hook = lambda h: _hook.__setitem__(0, h)
    mod.get_axon_ntff_profile_hook = lambda: _hook[0]
    _sys.modules["antenv.axon_hooks"] = mod
    antenv.axon_hooks = mod
    from trn_agent_boot.trn_boot import _ntff_profile_via_ctypes
    mod.set_axon_ntff_profile_hook(
        _ntff_profile_via_ctypes("/opt/axon/libaxon_pjrt.so"))


def run(inputs: dict, trace: bool = False):
    if trace:
        _ensure_ntff_hook()
    x = np.asarray(inputs["x"], dtype=np.float32)
    mask = np.asarray(inputs["mask"])
    apply_mask = not bool((mask == 1).all())
    nc = _get_nc(apply_mask)
    maps = _in_maps(x, mask, np.asarray(inputs["Wq"], np.float32),
                    np.asarray(inputs["bq"], np.float32),
                    np.asarray(inputs["Wk"], np.float32),
                    np.asarray(inputs["bk"], np.float32),
                    np.asarray(inputs["Wv"], np.float32),
                    np.asarray(inputs["bv"], np.float32), apply_mask)
    res = run_bass_kernel_spmd(nc, maps, core_ids=list(range(NCORES)), trace=trace)
    out = np.empty((B, S, HID), dtype=np.float32)
    for c in range(NCORES):
        b, hg = c // 4, c % 4
        cs = slice(hg * COLS, (hg + 1) * COLS)
        ctxT = res.results[c]["out"]          # [HPC, D, S]
        out[b, :, cs] = ctxT.transpose(2, 0, 1).reshape(S, COLS)
    return out, res


def kernel(**inputs) -> np.ndarray:
    out, _ = run(inputs)
    return out
